# revision 3
# baseline (speedup 1.0000x reference)
"""Attention-GRU decoder (teacher forcing) on 8 TRN2 NeuronCores.

Strategy (v2):
  Phase 0 (per core, batch sharded 4 seqs/core): big single-shot DMAs of all
     weights (recurrence weights in fp8e4, x256 scaled); precompute
     EcT_b  = enc @ W1_enc.T / 256 + b1   (attention enc projection + bias)
     EncWc  = enc @ Wc.T (fp8 storage)    (context->GRU-input, hoisted)
     GIX    = x_aug @ Wx_aug              (all-steps input proj + biases, fp16)
     A tiny warm-up AllGather at kernel start absorbs cross-core launch skew
     and first-collective setup off the critical path.
  Phase 1: 31 sequential steps. W1h/W_hh/EncWc stationaries are fp8 (FWL
     halves weight-load time); h stays fp16 moving. Attention pre-activation
     formed by ONE 4D-broadcast STT + ONE big tanh (instead of 8+8 small
     ops). Sigmoid eliminated via tanh(x/2) identity folded into fused
     scalar_tensor_tensor gate math -> phase 1 uses only the exp/tanh
     activation-table set (no per-step table reloads). Partial AllGathers of
     h every ~8 steps overlap compute; h lands as fp8 DoubleRow-packed
     stationaries for phase 2.
  Phase 2: vocab-parallel output projection in fp8 DoubleRow (2 contraction
     rows/cycle, half the matmuls), log-softmax via exp+accum, pipelined
     AllReduce of sum-exp, final log-probs emitted fp16 (cast to fp32 on
     host; fp16 quantization ~6e-4 rel, well under budget).

kernel(**inputs) takes full inputs, returns [B, T-1, V] float32.
"""
import numpy as np
import ml_dtypes

import concourse.bacc as bacc
import concourse.bass as bass
import concourse.mybir as mybir
import concourse.tile as tile
from concourse.bass_utils import run_bass_kernel_spmd

F32 = mybir.dt.float32
F16 = mybir.dt.float16
F8 = mybir.dt.float8e4
AF = mybir.ActivationFunctionType
ALU = mybir.AluOpType
DR = mybir.MatmulPerfMode.DoubleRow

B, S, H, V, Dw, T = 32, 50, 1024, 32000, 512, 32
NCORES = 8
P = 128
TS = T - 1            # 31 decode steps
BC = B // NCORES      # 4 sequences per core
VC = V // NCORES      # 4000 vocab rows per core
SP = 64               # padded s-block per sequence (partition alignment)
NBS = BC * SP         # 256 padded (b,s) columns per core
ROWS = TS * BC        # 124 hidden rows per rank
RTOT = TS * B         # 992 total rows
KH = H // P           # 8 hidden chunks
KG = 3 * H // P       # 24 gate chunks
KP = KH // 2          # 4 DoubleRow k-pair chunks
NV = 8                # vocab n-chunks per core
NVS = VC // NV        # 500
AG_CHUNKS = [(1, 9), (9, 17), (17, 25), (25, 32)]  # step-slot ranges gathered
SW = 256.0            # fp8 weight scale
ISW = 1.0 / SW

_CACHE = {}


def _build():
    nc = bacc.Bacc("TRN2", target_bir_lowering=False, debug=False,
                   num_devices=NCORES)

    def din(name, shape, dt):
        return nc.dram_tensor(name, shape, dt, kind="ExternalInput").ap()

    enct16_d = din("enct16", [P, KH, NBS], F16)
    w1et8_d = din("w1et8", [P, KH, H], F8)
    wct8_d = din("wct8", [P, KH, 3 * H], F8)
    wxat16_d = din("wxat16", [P, 5, 3 * H], F16)
    xat16_d = din("xat16", [P, 5, P], F16)
    whht8_d = din("whht8", [P, KH, 3 * H], F8)
    w1ht8_d = din("w1ht8", [P, KH, H], F8)
    w2t16_d = din("w2t16", [P, KH], F16)
    b1t_d = din("b1t", [P, KH], F32)
    bhnrep_d = din("bhnrep", [P, KH * BC], F32)
    h0t_d = din("h0t", [P, KH * BC], F32)
    wo8_d = din("wo8", [P, KP, 2, VC], F8)
    bout16_d = din("bout16", [1, VC], F16)
    out_d = nc.dram_tensor("out", [RTOT, VC], F16, kind="ExternalOutput").ap()

    rg = [list(range(NCORES))]

    with tile.TileContext(nc) as tc:
        with tc.tile_pool(name="dram", bufs=1, space="DRAM") as dram:
            # collective buffers
            agw_in = dram.tile([P, 1], F32, name="agw_in")
            agw_out = dram.tile([NCORES, P, 1], F32, name="agw_out")
            agin, agout = [], []
            for j, (tlo, thi) in enumerate(AG_CHUNKS):
                w = (thi - tlo) * BC
                agin.append(dram.tile([H, w], F32, name=f"agin{j}"))
                agout.append(dram.tile([NCORES, H, w], F32, name=f"agout{j}"))
            arin_p = [dram.tile([2 * ROWS, 1], F32, name=f"arin{p}")
                      for p in range(4)]
            arout_p = [dram.tile([2 * ROWS, 1], F32, name=f"arout{p}")
                       for p in range(4)]

            pwo_cm = tc.tile_pool(name="pwo", bufs=1)
            pwo = pwo_cm.__enter__()
            wo8 = pwo.tile([P, KP, 2, VC], F8)
            hga8 = pwo.tile([P, KP, 2, NCORES, P], F8)

            with tc.tile_pool(name="pw", bufs=1) as pw:
                # ---- tiles that live through phases 0+1 ----
                whht8 = pw.tile([P, KH, 3 * H], F8)
                w1ht8 = pw.tile([P, KH, H], F8)
                ecT = pw.tile([P, KH, NBS], F32)
                encwc8 = pw.tile([P, 2, 3 * H], F8)
                gixt = pw.tile([P, KG, TS, BC], F32)
                hallT = pw.tile([P, KH, T, BC], F32)
                aw16 = pw.tile([P, KH, NBS], F16)
                w2t16 = pw.tile([P, KH], F16)
                b1t = pw.tile([P, KH], F32)
                bhnrep = pw.tile([P, KH, BC], F32)
                ones1 = pw.tile([1, 1], F16)
                bd1 = pw.tile([P, BC], F16)
                bd2 = pw.tile([P, BC], F16)
                warm = pw.tile([P, 1], F32)

                # warm-up collective: absorbs launch skew + CC setup during
                # phase 0 (no consumers -> never blocks the critical path)
                nc.vector.memset(warm[:], 0.0)
                nc.sync.dma_start(out=agw_in[:], in_=warm[:])
                nc.gpsimd.collective_compute(
                    "AllGather", ALU.bypass, replica_groups=rg,
                    ins=[agw_in.opt()], outs=[agw_out.opt()])

                nc.sync.dma_start(out=w2t16[:], in_=w2t16_d[:])
                nc.sync.dma_start(out=b1t[:], in_=b1t_d[:])
                nc.sync.dma_start(
                    out=bhnrep[:],
                    in_=bhnrep_d[:].rearrange("p (k b) -> p k b", b=BC))
                nc.sync.dma_start(
                    out=hallT[:, :, 0, :],
                    in_=h0t_d[:].rearrange("p (k b) -> p k b", b=BC))
                nc.vector.memset(ones1[:], 1.0)
                nc.vector.memset(bd1[:], 0.0)
                nc.vector.memset(bd2[:], 0.0)
                nc.vector.memset(aw16[:], 0.0)
                nc.vector.memset(hga8[:], 0.0)

                # ---------------- phase 0 ----------------
                with tc.tile_pool(name="p0a", bufs=1) as p0a:
                    # all phase-0 weights land with single-shot DMAs
                    xat16 = p0a.tile([P, 5, P], F16)
                    wxat16 = p0a.tile([P, 5, 3 * H], F16)
                    w1et8 = p0a.tile([P, KH, H], F8)
                    wct8 = p0a.tile([P, KH, 3 * H], F8)
                    enct16 = p0a.tile([P, KH, NBS], F16)
                    nc.sync.dma_start(out=xat16[:], in_=xat16_d[:])
                    nc.sync.dma_start(out=wxat16[:], in_=wxat16_d[:])
                    nc.sync.dma_start(out=enct16[:], in_=enct16_d[:])
                    nc.sync.dma_start(out=w1et8[:], in_=w1et8_d[:])
                    nc.sync.dma_start(out=wct8[:], in_=wct8_d[:])
                    # recurrence + output weights behind them
                    nc.sync.dma_start(out=whht8[:], in_=whht8_d[:])
                    nc.sync.dma_start(out=w1ht8[:], in_=w1ht8_d[:])
                    nc.sync.dma_start(out=wo8[:], in_=wo8_d[:])

                    # GIX (fp16): 6 psum tiles of [P, 4, P]
                    with tc.tile_pool(name="ps_gx_pool", bufs=1,
                                      space="PSUM") as psgx:
                        ps_gx = [psgx.tile([P, 4, P], F32, name=f"ps_gx{g}")
                                 for g in range(6)]
                        for k in range(5):
                            for mo in range(KG):
                                nc.tensor.matmul(
                                    ps_gx[mo // 4][:, mo % 4, :],
                                    wxat16[:, k, mo * P:(mo + 1) * P],
                                    xat16[:, k, :], start=(k == 0),
                                    stop=(k == 4))
                        for mo in range(KG):
                            nc.scalar.copy(
                                gixt[:, mo, :, :],
                                ps_gx[mo // 4][:, mo % 4, 0:ROWS].rearrange(
                                    "p (t b) -> p t b", b=BC))

                    # EcT_b = enc @ (256 W1e).T / 256 + b1 (k-outer, 8 banks)
                    with tc.tile_pool(name="ps_ec_pool", bufs=1,
                                      space="PSUM") as psec:
                        ps_ec = [psec.tile([P, NBS], F32, name=f"ps_ec{mo}")
                                 for mo in range(KH)]
                        for k in range(KH):
                            for mo in range(KH):
                                nc.tensor.matmul(
                                    ps_ec[mo][:],
                                    w1et8[:, k, mo * P:(mo + 1) * P],
                                    enct16[:, k, :],
                                    start=(k == 0), stop=(k == KH - 1))
                        for mo in range(KH):
                            nc.scalar.activation(
                                ecT[:, mo, :], ps_ec[mo][:], AF.Identity,
                                bias=b1t[:, mo:mo + 1], scale=ISW)

                    # EncWc (fp8 out, unscaled): 6 n-chunks x 2 mt x 8 k
                    with tc.tile_pool(name="ps_ew_pool", bufs=2,
                                      space="PSUM") as psew:
                        for n in range(6):
                            nsl = slice(n * 512, (n + 1) * 512)
                            for mt in range(2):
                                ps_ew = psew.tile([P, 512], F32, name="ps_ew",
                                                  tag="ps_ew")
                                for k in range(KH):
                                    nc.tensor.matmul(
                                        ps_ew[:],
                                        enct16[:, k, mt * P:(mt + 1) * P],
                                        wct8[:, k, nsl],
                                        start=(k == 0), stop=(k == KH - 1))
                                nc.scalar.activation(
                                    encwc8[:, mt, nsl], ps_ew[:], AF.Copy,
                                    scale=ISW)

                # ---------------- phase 1: 31 steps ----------------
                with (
                    tc.tile_pool(name="p1", bufs=2) as p1,
                    tc.tile_pool(name="ps_hp_pool", bufs=1, space="PSUM") as pshp,
                    tc.tile_pool(name="ps_gh_pool", bufs=1, space="PSUM") as psgh,
                    tc.tile_pool(name="ps_gic_pool", bufs=1, space="PSUM") as psgic,
                    tc.tile_pool(name="ps_e_pool", bufs=1, space="PSUM") as pse,
                    tc.tile_pool(name="ps_a_pool", bufs=1, space="PSUM") as psa,
                ):
                    for t in range(1, T):
                        hprev = hallT[:, :, t - 1, :]
                        h16 = p1.tile([P, KH, BC], F16, name="h16", tag="h16")
                        nc.vector.tensor_copy(h16[:], hprev)

                        # Hproj (fp8 stationary, W1h x256)
                        ps_hp = pshp.tile([P, KH, BC], F32, name="ps_hp", tag="hp")
                        for mo in range(KH):
                            for k in range(KH):
                                nc.tensor.matmul(
                                    ps_hp[:, mo, :],
                                    w1ht8[:, k, mo * P:(mo + 1) * P],
                                    h16[:, k, :],
                                    start=(k == 0), stop=(k == KH - 1))

                        # gh (fp8 stationary, W_hh x256) -- the big one
                        ps_gh = psgh.tile([P, KG, BC], F32, name="ps_gh", tag="gh")
                        for mo in range(KG):
                            for k in range(KH):
                                nc.tensor.matmul(
                                    ps_gh[:, mo, :],
                                    whht8[:, k, mo * P:(mo + 1) * P],
                                    h16[:, k, :],
                                    start=(k == 0), stop=(k == KH - 1))

                        # attention: aw = tanh(EcT_b + Hproj/256) -- one STT,
                        # one big tanh
                        awp = p1.tile([P, KH, BC, SP], F32, name="awp",
                                      tag="awp")
                        nc.vector.scalar_tensor_tensor(
                            awp[:],
                            ps_hp[:].broadcast_to([P, KH, BC, SP]),
                            ISW,
                            ecT[:].rearrange("p k (b s) -> p k b s", s=SP),
                            op0=ALU.mult, op1=ALU.add)
                        nc.scalar.activation(
                            aw16[:].rearrange("p k (b s) -> p k b s", s=SP),
                            awp[:], AF.Tanh)

                        # e = w2 . aw  -> [1, bs]
                        ps_e = pse.tile([1, NBS], F32, name="ps_e", tag="e")
                        for k in range(KH):
                            nc.tensor.matmul(
                                ps_e[:], w2t16[:, k:k + 1], aw16[:, k, :],
                                start=(k == 0), stop=(k == KH - 1))

                        # softmax over s within each b (no max-shift: |e| small)
                        expe = p1.tile([1, NBS], F32, name="expe", tag="expe")
                        nc.scalar.activation(expe[:], ps_e[:], AF.Exp)
                        s4 = p1.tile([1, BC], F32, name="s4", tag="s4")
                        nc.vector.reduce_sum(
                            s4[:], expe[:].rearrange("a (b s) -> a b s", s=SP)
                            [:, :, 0:S],
                            axis=mybir.AxisListType.X)
                        r4 = p1.tile([1, BC], F32, name="r4", tag="r4")
                        nc.vector.reciprocal(r4[:], s4[:])
                        alphan = p1.tile([1, NBS], F16, name="alphan", tag="aln")
                        if t <= 2:
                            nc.vector.memset(alphan[:], 0.0)
                        for b in range(BC):
                            nc.vector.tensor_scalar_mul(
                                alphan[:, b * SP:b * SP + S],
                                expe[:, b * SP:b * SP + S], r4[:, b:b + 1])

                        # transpose alpha to partitions via K=1 matmuls
                        ps_a1 = psa.tile([P, 1], F32, name="ps_a1", tag="a1")
                        ps_a2 = psa.tile([P, 1], F32, name="ps_a2", tag="a2")
                        nc.tensor.matmul(ps_a1[:], alphan[:, 0:P], ones1[:],
                                         start=True, stop=True)
                        nc.tensor.matmul(ps_a2[:], alphan[:, P:NBS], ones1[:],
                                         start=True, stop=True)
                        nc.vector.tensor_copy(bd1[0:64, 0:1], ps_a1[0:64, :])
                        nc.vector.tensor_copy(bd1[64:128, 1:2], ps_a1[64:128, :])
                        nc.vector.tensor_copy(bd2[0:64, 2:3], ps_a2[0:64, :])
                        nc.vector.tensor_copy(bd2[64:128, 3:4], ps_a2[64:128, :])

                        # gi_c = blockdiag(alpha) applied to EncWc (fp8)
                        ps_gic = psgic.tile([P, KG, BC], F32, name="ps_gic",
                                            tag="gic")
                        for mo in range(KG):
                            nc.tensor.matmul(
                                ps_gic[:, mo, :],
                                encwc8[:, 0, mo * P:(mo + 1) * P],
                                bd1[:], start=True, stop=False)
                            nc.tensor.matmul(
                                ps_gic[:, mo, :],
                                encwc8[:, 1, mo * P:(mo + 1) * P],
                                bd2[:], start=False, stop=True)

                        # gates -- sigmoid-free:
                        #   t = tanh(0.5 (s1 + gh/256));  r,z = 0.5 t + 0.5
                        #   n = tanh(s1_n + 0.5 (t_r+1) hn); h' = n + 0.5(t_z+1)(h-n)
                        s1 = p1.tile([P, KG, BC], F32, name="s1", tag="s1")
                        nc.vector.tensor_add(s1[:], ps_gic[:],
                                             gixt[:, :, t - 1, :])
                        b2 = p1.tile([P, 2 * KH, BC], F32, name="b2", tag="b2")
                        nc.vector.scalar_tensor_tensor(
                            b2[:], ps_gh[:, 0:2 * KH, :], ISW,
                            s1[:, 0:2 * KH, :], op0=ALU.mult, op1=ALU.add)
                        tt = p1.tile([P, 2 * KH, BC], F32, name="tt", tag="tt")
                        nc.scalar.activation(tt[:], b2[:], AF.Tanh, scale=0.5)
                        hn = p1.tile([P, KH, BC], F32, name="hn", tag="hn")
                        nc.vector.scalar_tensor_tensor(
                            hn[:], ps_gh[:, 2 * KH:KG, :], ISW, bhnrep[:],
                            op0=ALU.mult, op1=ALU.add)
                        m1p = p1.tile([P, KH, BC], F32, name="m1p", tag="m1p")
                        nc.vector.scalar_tensor_tensor(
                            m1p[:], tt[:, 0:KH, :], 1.0, hn[:],
                            op0=ALU.add, op1=ALU.mult)
                        s3 = p1.tile([P, KH, BC], F32, name="s3", tag="s3")
                        nc.vector.scalar_tensor_tensor(
                            s3[:], m1p[:], 0.5, s1[:, 2 * KH:KG, :],
                            op0=ALU.mult, op1=ALU.add)
                        nn_t = p1.tile([P, KH, BC], F32, name="nn_t", tag="nn")
                        nc.scalar.activation(nn_t[:], s3[:], AF.Tanh)
                        dd = p1.tile([P, KH, BC], F32, name="dd", tag="dd")
                        nc.vector.tensor_sub(dd[:], hprev, nn_t[:])
                        e1 = p1.tile([P, KH, BC], F32, name="e1", tag="e1")
                        nc.vector.scalar_tensor_tensor(
                            e1[:], tt[:, KH:2 * KH, :], 1.0, dd[:],
                            op0=ALU.add, op1=ALU.mult)
                        nc.vector.scalar_tensor_tensor(
                            hallT[:, :, t, :], e1[:], 0.5, nn_t[:],
                            op0=ALU.mult, op1=ALU.add)

                        # partial allgather of finished h slots
                        for j, (tlo, thi) in enumerate(AG_CHUNKS):
                            if t == thi - 1:
                                for k in range(KH):
                                    nc.sync.dma_start(
                                        out=agin[j][k * P:(k + 1) * P, :]
                                            .rearrange("p (t b) -> p t b", b=BC),
                                        in_=hallT[:, k, tlo:thi, :])
                                nc.gpsimd.collective_compute(
                                    "AllGather", ALU.bypass, replica_groups=rg,
                                    ins=[agin[j].opt()], outs=[agout[j].opt()])
                                w_j = (thi - tlo) * BC
                                base = (tlo - 1) * BC
                                for k in range(KH):
                                    hgs = p1.tile([P, NCORES, 32], F32,
                                                  name="hgs", tag="hgs")
                                    nc.sync.dma_start(
                                        out=hgs[:, :, 0:w_j],
                                        in_=agout[j][:, k * P:(k + 1) * P, :]
                                            .rearrange("r p w -> p r w"))
                                    nc.vector.tensor_copy(
                                        hga8[:, k // 2, k % 2, :,
                                             base:base + w_j],
                                        hgs[:, :, 0:w_j])

            # ---------------- phase 2 ----------------
            with (
                tc.tile_pool(name="p2", bufs=1) as p2,
                tc.tile_pool(name="p2s", bufs=2) as p2s,
                tc.tile_pool(name="p2x", bufs=4) as p2x,
                tc.tile_pool(name="ps2", bufs=6, space="PSUM") as ps2,
                tc.tile_pool(name="ps2b", bufs=2, space="PSUM") as ps2b,
            ):
                # b_out broadcast to all partitions via K=1 ones matmul
                ones16 = p2.tile([1, P], F16)
                nc.vector.memset(ones16[:], 1.0)
                bout16 = p2.tile([1, VC], F16)
                nc.sync.dma_start(out=bout16[:], in_=bout16_d[:])
                boutrep = p2.tile([P, VC], F32)
                for n in range(NV):
                    ps_b = ps2b.tile([P, NVS], F32, name="ps_b", tag="ps_b")
                    nc.tensor.matmul(ps_b[:], ones16[:],
                                     bout16[:, n * NVS:(n + 1) * NVS],
                                     start=True, stop=True)
                    nc.scalar.copy(boutrep[:, n * NVS:(n + 1) * NVS], ps_b[:])

                lg_tiles = [None] * NCORES
                sums_tiles = [None] * NCORES

                def compute_m(m):
                    lg = p2.tile([ROWS, VC], F16, name="lg", tag="lg", bufs=8)
                    sums = p2s.tile([ROWS, NV], F32, name="sums", tag="sums")
                    lg_tiles[m] = lg
                    sums_tiles[m] = sums
                    for n in range(NV):
                        nsl = slice(n * NVS, (n + 1) * NVS)
                        ps_o = ps2.tile([P, NVS], F32, name="ps_o", tag="ps_o")
                        for kp in range(KP):
                            nc.tensor.matmul(
                                ps_o[:], hga8[:, kp, :, m, :],
                                wo8[:, kp, :, nsl], perf_mode=DR,
                                start=(kp == 0), stop=(kp == KP - 1))
                        nc.vector.scalar_tensor_tensor(
                            lg[:, nsl], ps_o[0:ROWS, :], ISW,
                            boutrep[0:ROWS, nsl], op0=ALU.mult, op1=ALU.add)
                        etrash = p2x.tile([ROWS, NVS], F16, name="etrash",
                                          tag="et")
                        nc.scalar.activation(etrash[:], lg[:, nsl], AF.Exp,
                                             accum_out=sums[:, n:n + 1])
                    ssum = p2s.tile([ROWS, 1], F32, name="ssum", tag="ssum")
                    nc.vector.reduce_sum(ssum[:], sums[:],
                                         axis=mybir.AxisListType.X)
                    nc.sync.dma_start(
                        out=arin_p[m // 2][(m % 2) * ROWS:(m % 2 + 1) * ROWS, :],
                        in_=ssum[:])

                def emit_ar(p):
                    nc.gpsimd.collective_compute(
                        "AllReduce", ALU.add, replica_groups=rg,
                        ins=[arin_p[p].opt()], outs=[arout_p[p].opt()])

                def finalize_pair(p):
                    for m in (2 * p, 2 * p + 1):
                        lz = p2s.tile([ROWS, 1], F32, name="lz", tag="lz")
                        nc.sync.dma_start(
                            out=lz[:],
                            in_=arout_p[p][(m % 2) * ROWS:(m % 2 + 1) * ROWS, :])
                        lzl = p2s.tile([ROWS, 1], F32, name="lzl", tag="lzl")
                        nc.scalar.activation(lzl[:], lz[:], AF.Ln, scale=1.0)
                        ostage = p2x.tile([ROWS, VC], F16, name="ostage",
                                          tag="os", bufs=2)
                        half = VC // 2
                        for hh in range(2):
                            hsl = slice(hh * half, (hh + 1) * half)
                            nc.vector.tensor_scalar(
                                ostage[:, hsl], lg_tiles[m][:, hsl],
                                lzl[:, 0:1], None, op0=ALU.subtract)
                        nc.gpsimd.dma_start(
                            out=out_d[m * ROWS:(m + 1) * ROWS, :],
                            in_=ostage[:])

                # software-pipelined emission: finalize(p) goes out two
                # compute-pairs after its AllReduce so it never stalls a queue
                for m in range(NCORES):
                    compute_m(m)
                    if m % 2 == 1:
                        emit_ar(m // 2)
                        if m >= 3:
                            finalize_pair(m // 2 - 1)
                finalize_pair(3)

            pwo_cm.__exit__(None, None, None)

    nc.compile()
    return nc


def _t8(w, nk=8):
    # [nk*128, M] -> [128, nk, M]
    m = w.shape[1]
    return np.ascontiguousarray(w.reshape(nk, P, m).transpose(1, 0, 2))


def _f8(x):
    return np.clip(x, -240.0, 240.0).astype(ml_dtypes.float8_e4m3)


def _prep_inputs(inputs):
    enc = np.asarray(inputs["encoder_outputs"], np.float32)
    ehid = np.asarray(inputs["encoder_hidden"], np.float32)
    targets = np.asarray(inputs["targets"])
    emb = np.asarray(inputs["emb"], np.float32)
    W1 = np.asarray(inputs["attn_W1"], np.float32)
    b1 = np.asarray(inputs["attn_b1"], np.float32)
    W2 = np.asarray(inputs["attn_W2"], np.float32)
    W_ih = np.asarray(inputs["W_ih"], np.float32)
    b_ih = np.asarray(inputs["b_ih"], np.float32)
    W_hh = np.asarray(inputs["W_hh"], np.float32)
    b_hh = np.asarray(inputs["b_hh"], np.float32)
    W_out = np.asarray(inputs["W_out"], np.float32)
    b_out = np.asarray(inputs["b_out"], np.float32)

    # shared (replicated across cores); fp8 weights carry a x256 scale
    w1et8 = _f8(_t8(W1[:, :H].T) * SW)
    w1ht8 = _f8(_t8(np.ascontiguousarray(W1[:, H:]).T) * SW)
    wct8 = _f8(_t8(np.ascontiguousarray(W_ih[:, Dw:]).T) * SW)
    whht8 = _f8(_t8(W_hh.T) * SW)
    wxa = np.zeros((640, 3 * H), np.float32)
    wxa[:Dw] = W_ih[:, :Dw].T
    wxa[Dw] = b_ih + np.concatenate([b_hh[:2 * H], np.zeros(H, np.float32)])
    wxat16 = _t8(wxa, nk=5).astype(np.float16)
    w2t16 = np.ascontiguousarray(W2[0].reshape(KH, P).T).astype(np.float16)
    b1t = np.ascontiguousarray(b1.reshape(KH, P).T)
    bhnrep = np.ascontiguousarray(
        np.repeat(b_hh[2 * H:].reshape(KH, P).T[:, :, None], BC, axis=2)
        .reshape(P, KH * BC))

    x_all = emb[targets[:, :TS]]  # [B, TS, Dw]

    in_maps = []
    for c in range(NCORES):
        bsl = slice(c * BC, (c + 1) * BC)
        vsl = slice(c * VC, (c + 1) * VC)
        encT = np.zeros((H, BC, SP), np.float32)
        encT[:, :, :S] = enc[bsl].transpose(2, 0, 1)
        enct16 = _t8(encT.reshape(H, NBS)).astype(np.float16)
        xat = np.zeros((640, P), np.float32)
        xat[:Dw, :ROWS] = x_all[bsl].transpose(2, 1, 0).reshape(Dw, ROWS)
        xat[Dw, :ROWS] = 1.0
        xat16 = _t8(xat, nk=5).astype(np.float16)
        h0t = np.ascontiguousarray(
            ehid[0, bsl].T.reshape(KH, P, BC).transpose(1, 0, 2)
            .reshape(P, KH * BC))
        # DoubleRow W_out: wo8[p, kp, j, n] = 256 W_out[vsl][n, (2kp+j)128+p]
        wo8 = _f8(np.ascontiguousarray(
            (W_out[vsl].T * SW).reshape(KP, 2, P, VC).transpose(2, 0, 1, 3)))
        bout16 = np.ascontiguousarray(b_out[vsl][None, :]).astype(np.float16)
        in_maps.append({
            "enct16": enct16, "w1et8": w1et8, "wct8": wct8,
            "wxat16": wxat16, "xat16": xat16, "whht8": whht8, "w1ht8": w1ht8,
            "w2t16": w2t16, "b1t": b1t, "bhnrep": bhnrep, "h0t": h0t,
            "wo8": wo8, "bout16": bout16,
        })
    return in_maps


def kernel(**inputs):
    if "nc" not in _CACHE:
        _CACHE["nc"] = _build()
    nc = _CACHE["nc"]
    in_maps = _prep_inputs(inputs)
    res = run_bass_kernel_spmd(nc, in_maps, core_ids=list(range(NCORES)))
    L = np.stack([np.asarray(res.results[c]["out"], np.float32)
                  for c in range(NCORES)])
    L = (L.reshape(NCORES, NCORES, TS, BC, VC)
         .transpose(1, 3, 2, 0, 4).reshape(B, TS, V))
    return np.ascontiguousarray(L, dtype=np.float32)
